# revision 11
# baseline (speedup 1.0000x reference)
"""AConnect (nn_AConnect_82368882803074) Trainium2 kernel.

Reference computation:
    memW[b]    = W * Werr_bank[idx[b]]             [B, D_in, D_out]
    membias[b] = bias * Berr_bank[idx[b]]          [B, 1, D_out]
    Z[b]       = X[b] @ memW[b] + membias[b]       [B, D_out]

Strategy: data-parallel over the batch across 8 NeuronCores, with
duplicate-bank dedup. The host groups samples by bank index and packs the
banks onto cores ("slots"); each slot loads its bank matrix once and carries
up to M=4 samples as extra matmul columns. The host only moves data (gather,
transpose, zero-padding, output permutation); all arithmetic (W ⊙ E,
X @ (W ⊙ E), bias ⊙ Berr and the final add) runs on device.

Per core the device kernel streams K gathered 1 MB bank matrices from HBM,
casting f32->bf16 inside the (SWDGE) DMA, multiplies by W on VectorE in bf16
(2x mode), and contracts with the slot's 4 X-columns on TensorE (4 k-chunk
matmuls accumulating into a [4, 512] PSUM tile). PSUM rows are staged via
ScalarE into a partition-0..3 strip, reshaped by one SBUF->SBUF DMA, the
bias term is added, and one DMA writes the slot-ordered output.
"""

import numpy as np

B, D_IN, D_OUT, N_BANK, N_CORES = 256, 512, 512, 1000, 8
P = 128  # partitions
C = D_IN // P  # 4 k-chunks
M = 4  # samples per bank slot (max observed bank multiplicity is 3)

_CACHE = {}
last_exec_time_ns = None


def _build_nc(K):
    """Device graph for K bank-slots per core."""
    import concourse.mybir as mybir
    import concourse.tile as tile
    from concourse import bacc

    f32 = mybir.dt.float32
    bf16 = mybir.dt.bfloat16
    nc = bacc.Bacc()

    R = K * M  # output rows (slot-major)
    eg = nc.dram_tensor("eg", [K, P, C * D_OUT], f32, kind="ExternalInput")
    wt = nc.dram_tensor("wt", [P, C * D_OUT], f32, kind="ExternalInput")
    xtt = nc.dram_tensor("xtt", [P, C * K * M], f32, kind="ExternalInput")
    bb = nc.dram_tensor("bb", [R, D_OUT], f32, kind="ExternalInput")
    beg = nc.dram_tensor("beg", [R, D_OUT], f32, kind="ExternalInput")
    out = nc.dram_tensor("out", [R, D_OUT], f32, kind="ExternalOutput")

    with tile.TileContext(nc) as tc:
        with (
            tc.tile_pool(name="const", bufs=1) as constp,
            tc.tile_pool(name="ep", bufs=10) as ep,
            tc.tile_pool(name="wep", bufs=6) as wep,
            tc.tile_pool(name="ps", bufs=6, space="PSUM") as psp,
            tc.tile_pool(name="outp", bufs=1) as outp,
        ):
            w_t = constp.tile([P, C * D_OUT], f32)
            nc.sync.dma_start(w_t[:], wt[:])
            x_t = constp.tile([P, C * K * M], f32)
            nc.sync.dma_start(x_t[:], xtt[:])
            bias_t = constp.tile([R, D_OUT], f32)
            nc.sync.dma_start(bias_t[:], bb[:])
            berr_t = constp.tile([R, D_OUT], f32)
            nc.sync.dma_start(berr_t[:], beg[:])
            mb = constp.tile([R, D_OUT], f32)
            nc.vector.tensor_mul(mb[:], bias_t[:], berr_t[:])

            # bf16 copies of the resident matmul operands
            w_b = constp.tile([P, C * D_OUT], bf16)
            nc.vector.tensor_copy(w_b[:], w_t[:])
            x_b = constp.tile([P, C * K * M], bf16)
            nc.vector.tensor_copy(x_b[:], x_t[:])

            zstage = outp.tile([M, K * D_OUT], f32)
            for t in range(K):
                # f32 -> bf16 cast happens inside the (SWDGE) DMA
                eb = ep.tile([P, C * D_OUT], bf16)
                nc.gpsimd.dma_start(eb[:], eg[t])
                we = wep.tile([P, C * D_OUT], bf16)
                nc.vector.tensor_mul(we[:], eb[:], w_b[:])
                ps = psp.tile([M, D_OUT], f32)
                for c in range(C):
                    nc.tensor.matmul(
                        ps[:],
                        x_b[:, (c * K + t) * M : (c * K + t) * M + M],
                        we[:, c * D_OUT : (c + 1) * D_OUT],
                        start=(c == 0),
                        stop=(c == C - 1),
                    )
                nc.any.tensor_copy(zstage[0:M, t * D_OUT : (t + 1) * D_OUT], ps[:])

            # zre rows are j-major: row j*K + t holds slot t, column j.
            # (an SBUF AP may only have one partition dim, so the reshape
            # DMA iterates (j, t, n) and the host uses the same row order)
            zre = outp.tile([R, D_OUT], f32)
            nc.sync.dma_start(
                zre[:],
                zstage[0:M, :].rearrange("j (t n) -> j t n", n=D_OUT),
            )
            fin = outp.tile([R, D_OUT], f32)
            nc.vector.tensor_add(fin[:], zre[:], mb[:])
            nc.sync.dma_start(out[:], fin[:])

    nc.compile()
    return nc


def _pack(idx):
    """Group samples by bank, pack banks onto cores.

    Returns (K, plan) where plan[c] is a list of (bank, [samples]) slots,
    each slot carrying at most M samples of one bank.
    """
    from collections import defaultdict

    groups = defaultdict(list)
    for s, b in enumerate(idx):
        groups[int(b)].append(s)
    # one slot per <=M samples of a bank
    slots = []
    for b, ss in groups.items():
        for i in range(0, len(ss), M):
            slots.append((b, ss[i : i + M]))
    slots.sort(key=lambda x: -len(x[1]))
    per_core = (len(slots) + N_CORES - 1) // N_CORES
    plan = [[] for _ in range(N_CORES)]
    order = sorted(range(N_CORES), key=lambda c: c)
    i = 0
    for b, ss in slots:
        # round-robin into the core with fewest slots
        c = min(range(N_CORES), key=lambda c: len(plan[c]))
        plan[c].append((b, ss))
        i += 1
    K = max(len(p) for p in plan)
    return K, plan


def _install_trace_shim():
    """Register the axon NTFF profile hook bass_utils expects (the agent
    image lacks antenv.axon_hooks; the C ABI is in libaxon_pjrt.so)."""
    import contextlib
    import ctypes
    import sys
    import types

    if "antenv.axon_hooks" in sys.modules:
        return
    lib = ctypes.CDLL("/opt/axon/libaxon_pjrt.so")
    if not hasattr(lib, "axon_start_nrt_profile"):
        hook = None
    else:
        lib.axon_start_nrt_profile.argtypes = [
            ctypes.POINTER(ctypes.c_int64),
            ctypes.c_size_t,
        ]
        lib.axon_start_nrt_profile.restype = ctypes.c_int64
        lib.axon_stop_nrt_profile.argtypes = [ctypes.c_char_p]
        lib.axon_stop_nrt_profile.restype = ctypes.c_int64

        @contextlib.contextmanager
        def hook(output_dir, device_ids):
            import jax

            jax.devices()
            if device_ids:
                ids = (ctypes.c_int64 * len(device_ids))(*device_ids)
                rc = lib.axon_start_nrt_profile(ids, len(device_ids))
            else:
                rc = lib.axon_start_nrt_profile(None, 0)
            if rc != 0:
                raise RuntimeError(f"axon_start_nrt_profile rc={rc}")
            try:
                yield
            finally:
                n = lib.axon_stop_nrt_profile(str(output_dir).encode())
                print(f"ntff profile: {n} file(s) -> {output_dir}", file=sys.stderr)

    mod = types.ModuleType("antenv.axon_hooks")
    mod.get_axon_ntff_profile_hook = lambda: hook
    mod.set_axon_ntff_profile_hook = lambda h: None
    sys.modules["antenv.axon_hooks"] = mod


def kernel(X, W, bias, Werr_bank, Berr_bank, idx):
    global last_exec_time_ns
    import os

    from concourse.bass_utils import run_bass_kernel_spmd

    X = np.asarray(X, dtype=np.float32)
    W = np.asarray(W, dtype=np.float32)
    bias = np.asarray(bias, dtype=np.float32)
    Werr_bank = np.asarray(Werr_bank, dtype=np.float32)
    Berr_bank = np.asarray(Berr_bank, dtype=np.float32)
    idx = np.asarray(idx, dtype=np.int32)

    K, plan = _pack(idx)
    if ("nc", K) not in _CACHE:
        _CACHE[("nc", K)] = _build_nc(K)
    nc = _CACHE[("nc", K)]
    R = K * M

    # Host-side sharding / layout (pure data movement).
    wt = np.ascontiguousarray(
        W.reshape(C, P, D_OUT).transpose(1, 0, 2).reshape(P, C * D_OUT)
    )
    bb = np.ascontiguousarray(np.broadcast_to(bias.reshape(1, D_OUT), (R, D_OUT)))

    in_maps = []
    row_of_sample = np.full(B, -1, dtype=np.int64)  # (core, row) flattened
    for c_id in range(N_CORES):
        slots = plan[c_id]
        banks = [b for b, _ in slots] + [0] * (K - len(slots))
        eg = Werr_bank[banks]  # [K, D_in, D_out]
        eg = np.ascontiguousarray(
            eg.reshape(K, C, P, D_OUT).transpose(0, 2, 1, 3).reshape(K, P, C * D_OUT)
        )
        # X columns in slot-major order: xs[t*M+j] = X[sample(t,j)] or 0
        xs = np.zeros((R, D_IN), dtype=np.float32)
        beg = np.zeros((R, D_OUT), dtype=np.float32)
        for t, (b, ss) in enumerate(slots):
            for j, s in enumerate(ss):
                xs[t * M + j] = X[s]
                # output rows are j-major (see zre comment in _build_nc)
                beg[j * K + t] = Berr_bank[b, 0]
                row_of_sample[s] = c_id * R + j * K + t
        xtt = np.ascontiguousarray(
            xs.T.reshape(C, P, R).transpose(1, 0, 2).reshape(P, C * R)
        )
        in_maps.append({"eg": eg, "wt": wt, "xtt": xtt, "bb": bb, "beg": beg})
    assert (row_of_sample >= 0).all()

    trace = os.environ.get("BASS_KERNEL_TRACE") == "1"
    if trace:
        _install_trace_shim()
    res = run_bass_kernel_spmd(
        nc,
        in_maps,
        core_ids=list(range(N_CORES)),
        trace=trace,
        trace_cores=[0] if trace else None,
    )
    last_exec_time_ns = res.exec_time_ns
    allrows = np.concatenate([r["out"] for r in res.results], axis=0)  # [8*R, 512]
    return np.ascontiguousarray(allrows[row_of_sample])


# revision 27
# speedup vs baseline: 1.2210x; 1.2210x over previous
"""AConnect (nn_AConnect_82368882803074) Trainium2 kernel.

Reference computation:
    memW[b]    = W * Werr_bank[idx[b]]             [B, D_in, D_out]
    membias[b] = bias * Berr_bank[idx[b]]          [B, 1, D_out]
    Z[b]       = X[b] @ memW[b] + membias[b]       [B, D_out]

Strategy: data-parallel over the batch across 8 NeuronCores, with
duplicate-bank dedup. The host groups samples by bank index and packs the
banks onto cores ("slots"); each slot loads its bank matrix once and carries
up to M=4 samples as extra matmul columns. The host only moves data (gather,
transpose, zero-padding, output permutation); all arithmetic (W ⊙ E,
X @ (W ⊙ E), bias ⊙ Berr and the final add) runs on device.

Per core the device kernel streams K gathered 1 MB bank matrices from HBM,
casting f32->bf16 inside the (SWDGE) DMA, multiplies by W on VectorE in bf16
(2x mode), and contracts with the slot's 4 X-columns on TensorE (4 k-chunk
matmuls accumulating into a [4, 512] PSUM tile). PSUM rows are staged via
ScalarE into a partition-0..3 strip, reshaped by one SBUF->SBUF DMA, the
bias term is added, and one DMA writes the slot-ordered output.
"""

import numpy as np

B, D_IN, D_OUT, N_BANK, N_CORES = 256, 512, 512, 1000, 8
P = 128  # partitions
C = D_IN // P  # 4 k-chunks
M = 4  # samples per bank slot (max observed bank multiplicity is 3)

_CACHE = {}
last_exec_time_ns = None


def _build_nc(K):
    """Device graph for K bank-slots per core."""
    import concourse.mybir as mybir
    import concourse.tile as tile
    from concourse import bacc

    f32 = mybir.dt.float32
    bf16 = mybir.dt.bfloat16
    nc = bacc.Bacc()

    Q = (K + 3) // 4  # slots per assembly quarter
    QB = ((M * Q + 31) // 32) * 32  # row block per quarter (32-aligned)
    NQ = (K + Q - 1) // Q
    R = NQ * QB  # output rows (quarter-blocked)
    eg = nc.dram_tensor("eg", [K, P, C * D_OUT], f32, kind="ExternalInput")
    wt = nc.dram_tensor("wt", [P, C * D_OUT], f32, kind="ExternalInput")
    xtt = nc.dram_tensor("xtt", [P, C * K * M], f32, kind="ExternalInput")
    bb = nc.dram_tensor("bb", [R, D_OUT], f32, kind="ExternalInput")
    beg = nc.dram_tensor("beg", [R, D_OUT], f32, kind="ExternalInput")
    out = nc.dram_tensor("out", [R, D_OUT], f32, kind="ExternalOutput")

    with tile.TileContext(nc) as tc:
        with (
            tc.tile_pool(name="const", bufs=1) as constp,
            tc.tile_pool(name="ep", bufs=10) as ep,
            tc.tile_pool(name="wep", bufs=6) as wep,
            tc.tile_pool(name="ps", bufs=6, space="PSUM") as psp,
            tc.tile_pool(name="outp", bufs=1) as outp,
        ):
            w_t = constp.tile([P, C * D_OUT], f32)
            nc.sync.dma_start(w_t[:], wt[:])
            x_t = constp.tile([P, C * K * M], f32)
            nc.sync.dma_start(x_t[:], xtt[:])
            # per-quarter bias-term tiles (TensorTensor operands must share
            # their base partition, so each quarter gets base-0 tiles)
            mbs = []
            for q in range((K + ((K + 3) // 4) - 1) // ((K + 3) // 4)):
                bias_q = constp.tile([QB, D_OUT], f32, name=f"bias{q}")
                nc.sync.dma_start(bias_q[:], bb[q * QB : (q + 1) * QB, :])
                berr_q = constp.tile([QB, D_OUT], f32, name=f"berr{q}")
                nc.sync.dma_start(berr_q[:], beg[q * QB : (q + 1) * QB, :])
                mb_q = constp.tile([QB, D_OUT], f32, name=f"mb{q}")
                nc.vector.tensor_mul(mb_q[:], bias_q[:], berr_q[:])
                mbs.append(mb_q)

            # bf16 copies of the resident matmul operands
            w_b = constp.tile([P, C * D_OUT], bf16)
            nc.vector.tensor_copy(w_b[:], w_t[:])
            x_b = constp.tile([P, C * K * M], bf16)
            nc.vector.tensor_copy(x_b[:], x_t[:])

            # Output assembly is pipelined in quarters of Q slots so the
            # partition-0..3 -> many-row reshape DMA overlaps the main loop.
            # Row order within a quarter is j-major: row q*QB + j*qs + i
            # holds slot t = q*Q + i, column j. (An SBUF AP may only have
            # one partition dim, so the reshape DMA iterates (j, i, n) and
            # the host uses the same row order. Quarter blocks are padded
            # to 32 rows because engine APs must start 32-aligned.)
            qsizes = [min(Q, K - q * Q) for q in range(NQ)]
            zstages = [
                outp.tile([M, qs * D_OUT], f32, name=f"zs{q}", tag=f"zs{q}")
                for q, qs in enumerate(qsizes)
            ]
            for t in range(K):
                # f32 -> bf16 cast happens inside the (SWDGE) DMA
                eb = ep.tile([P, C * D_OUT], bf16)
                nc.gpsimd.dma_start(eb[:], eg[t])
                we = wep.tile([P, C * D_OUT], bf16)
                nc.vector.tensor_mul(we[:], eb[:], w_b[:])
                ps = psp.tile([M, D_OUT], f32)
                for c in range(C):
                    nc.tensor.matmul(
                        ps[:],
                        x_b[:, (c * K + t) * M : (c * K + t) * M + M],
                        we[:, c * D_OUT : (c + 1) * D_OUT],
                        start=(c == 0),
                        stop=(c == C - 1),
                    )
                q, i = t // Q, t % Q
                nc.any.tensor_copy(
                    zstages[q][0:M, i * D_OUT : (i + 1) * D_OUT], ps[:]
                )
                if i == qsizes[q] - 1:
                    # quarter complete: reshape, add bias term, store
                    qs = qsizes[q]
                    r0 = q * QB
                    zre = outp.tile([M * qs, D_OUT], f32, name=f"zr{q}", tag=f"zr{q}")
                    nc.sync.dma_start(
                        zre[:],
                        zstages[q][0:M, :].rearrange("j (i n) -> j i n", n=D_OUT),
                    )
                    fin = outp.tile([M * qs, D_OUT], f32, name=f"fn{q}", tag=f"fn{q}")
                    nc.vector.tensor_add(fin[:], zre[:], mbs[q][0 : M * qs, :])
                    nc.sync.dma_start(out[r0 : r0 + M * qs, :], fin[:])

    nc.compile()
    return nc


def _pack(idx):
    """Group samples by bank, pack banks onto cores.

    Returns (K, plan) where plan[c] is a list of (bank, [samples]) slots,
    each slot carrying at most M samples of one bank.
    """
    from collections import defaultdict

    groups = defaultdict(list)
    for s, b in enumerate(idx):
        groups[int(b)].append(s)
    # one slot per <=M samples of a bank
    slots = []
    for b, ss in groups.items():
        for i in range(0, len(ss), M):
            slots.append((b, ss[i : i + M]))
    slots.sort(key=lambda x: -len(x[1]))
    per_core = (len(slots) + N_CORES - 1) // N_CORES
    plan = [[] for _ in range(N_CORES)]
    order = sorted(range(N_CORES), key=lambda c: c)
    i = 0
    for b, ss in slots:
        # round-robin into the core with fewest slots
        c = min(range(N_CORES), key=lambda c: len(plan[c]))
        plan[c].append((b, ss))
        i += 1
    K = max(len(p) for p in plan)
    return K, plan


def _install_trace_shim():
    """Register the axon NTFF profile hook bass_utils expects (the agent
    image lacks antenv.axon_hooks; the C ABI is in libaxon_pjrt.so)."""
    import contextlib
    import ctypes
    import sys
    import types

    if "antenv.axon_hooks" in sys.modules:
        return
    lib = ctypes.CDLL("/opt/axon/libaxon_pjrt.so")
    if not hasattr(lib, "axon_start_nrt_profile"):
        hook = None
    else:
        lib.axon_start_nrt_profile.argtypes = [
            ctypes.POINTER(ctypes.c_int64),
            ctypes.c_size_t,
        ]
        lib.axon_start_nrt_profile.restype = ctypes.c_int64
        lib.axon_stop_nrt_profile.argtypes = [ctypes.c_char_p]
        lib.axon_stop_nrt_profile.restype = ctypes.c_int64

        @contextlib.contextmanager
        def hook(output_dir, device_ids):
            import jax

            jax.devices()
            if device_ids:
                ids = (ctypes.c_int64 * len(device_ids))(*device_ids)
                rc = lib.axon_start_nrt_profile(ids, len(device_ids))
            else:
                rc = lib.axon_start_nrt_profile(None, 0)
            if rc != 0:
                raise RuntimeError(f"axon_start_nrt_profile rc={rc}")
            try:
                yield
            finally:
                n = lib.axon_stop_nrt_profile(str(output_dir).encode())
                print(f"ntff profile: {n} file(s) -> {output_dir}", file=sys.stderr)

    mod = types.ModuleType("antenv.axon_hooks")
    mod.get_axon_ntff_profile_hook = lambda: hook
    mod.set_axon_ntff_profile_hook = lambda h: None
    sys.modules["antenv.axon_hooks"] = mod


def kernel(X, W, bias, Werr_bank, Berr_bank, idx):
    global last_exec_time_ns
    import os

    from concourse.bass_utils import run_bass_kernel_spmd

    X = np.asarray(X, dtype=np.float32)
    W = np.asarray(W, dtype=np.float32)
    bias = np.asarray(bias, dtype=np.float32)
    Werr_bank = np.asarray(Werr_bank, dtype=np.float32)
    Berr_bank = np.asarray(Berr_bank, dtype=np.float32)
    idx = np.asarray(idx, dtype=np.int32)

    K, plan = _pack(idx)
    if ("nc", K) not in _CACHE:
        _CACHE[("nc", K)] = _build_nc(K)
    nc = _CACHE[("nc", K)]
    _Q = (K + 3) // 4
    R = ((K + _Q - 1) // _Q) * (((M * _Q + 31) // 32) * 32)

    # Host-side sharding / layout (pure data movement).
    wt = np.ascontiguousarray(
        W.reshape(C, P, D_OUT).transpose(1, 0, 2).reshape(P, C * D_OUT)
    )
    bb = np.ascontiguousarray(np.broadcast_to(bias.reshape(1, D_OUT), (R, D_OUT)))

    in_maps = []
    row_of_sample = np.full(B, -1, dtype=np.int64)  # (core, row) flattened
    for c_id in range(N_CORES):
        slots = plan[c_id]
        banks = [b for b, _ in slots] + [0] * (K - len(slots))
        eg = Werr_bank[banks]  # [K, D_in, D_out]
        eg = np.ascontiguousarray(
            eg.reshape(K, C, P, D_OUT).transpose(0, 2, 1, 3).reshape(K, P, C * D_OUT)
        )
        # X columns in slot-major order: xs[t*M+j] = X[sample(t,j)] or 0
        xs = np.zeros((K * M, D_IN), dtype=np.float32)
        beg = np.zeros((R, D_OUT), dtype=np.float32)
        Q = (K + 3) // 4
        QB = ((M * Q + 31) // 32) * 32
        for t, (b, ss) in enumerate(slots):
            q, i = t // Q, t % Q
            qs = min(Q, K - q * Q)
            for j, s in enumerate(ss):
                xs[t * M + j] = X[s]
                # output rows are quarter- then j-major (see _build_nc)
                r = q * QB + j * qs + i
                beg[r] = Berr_bank[b, 0]
                row_of_sample[s] = c_id * R + r
        xtt = np.ascontiguousarray(
            xs.T.reshape(C, P, K * M).transpose(1, 0, 2).reshape(P, C * K * M)
        )
        in_maps.append({"eg": eg, "wt": wt, "xtt": xtt, "bb": bb, "beg": beg})
    assert (row_of_sample >= 0).all()

    trace = os.environ.get("BASS_KERNEL_TRACE") == "1"
    if trace:
        _install_trace_shim()
    res = run_bass_kernel_spmd(
        nc,
        in_maps,
        core_ids=list(range(N_CORES)),
        trace=trace,
        trace_cores=[0] if trace else None,
    )
    last_exec_time_ns = res.exec_time_ns
    allrows = np.concatenate([r["out"] for r in res.results], axis=0)  # [8*R, 512]
    return np.ascontiguousarray(allrows[row_of_sample])
